# revision 34
# baseline (speedup 1.0000x reference)
"""Trainium2 Bass kernel for nn_MultiHeadAttention_83863531421896.

Full-input contract: kernel(**inputs) takes the unsharded tensors and
returns the full (2, 2048, 1024) output. Internally the 16 heads are
sharded 2-per-core across 8 NeuronCores (tensor parallel); each core
computes its heads' attention plus its slice of the output projection,
and the 8 partial projections are reduced on the host.

v4 design notes:
  - All matmul operands are bf16 (1 cycle/row on HW vs 2 for fp32),
    accumulation stays fp32 in PSUM. Host pre-casts x/weights; on-device
    casts are folded into existing PSUM->SBUF evictions for free.
  - exp runs on the Scalar engine in [128, 1024] tiles (both heads of a
    key-tile in one activation); the scalar queue carries only exps.
  - Minimal head: only K/V/Q-chunk0 of batch 0 (24 matmuls + 4
    transposes) run before attention; x DMA is ordered chunk-major so
    the first chunk lands ~3us in. Everything else (remaining K/V/Q
    chunks of b0, all of qkv(b1), both output projections) is injected
    instruction-by-instruction into attention's tensor-queue slots.
  - Softmax denominators: the AV matmul's ones-column gives per-query
    sums on PSUM partition 64; a 1-contraction-row matmul broadcasts
    that row to partitions 0..63 (no DRAM round-trip), then
    reciprocal_approx_fast + multiply normalize straight out of PSUM.
  - Partial output projection is written in bf16 (halves the out-DMA),
    summed across cores on the host in float64.
"""

import sys

if "/opt/trn_rl_repo" not in sys.path:
    sys.path.insert(0, "/opt/trn_rl_repo")

import numpy as np

B = 2
S = 2048
D = 1024
H = 16
HD = 64
N_CORES = 8
HEADS_PER_CORE = H // N_CORES  # 2
M = B * S                      # 4096 tokens
N_MCHUNK_B = S // 512          # 4 m-chunks of 512 tokens per batch
N_KTILE = D // 128             # 8 contraction tiles for qkv
N_QCHUNK = S // 512            # 4 q-chunks per batch
N_KKTILE = S // 128            # 16 key tiles per batch
SCALE = 1.0 / np.sqrt(HD)

_CACHE = {}


def _build_module():
    import concourse.bass as bass
    import concourse.tile as tile
    from concourse import bacc, mybir
    from concourse.bass import _add_dep_helper

    f32 = mybir.dt.float32
    bf16 = mybir.dt.bfloat16
    Exp = mybir.ActivationFunctionType.Exp

    nc = bacc.Bacc("TRN2", target_bir_lowering=False, debug=False,
                   num_devices=N_CORES)

    xt_ap = nc.dram_tensor("xt", [D, M], bf16, kind="ExternalInput").ap()
    wqa_ap = nc.dram_tensor("wqa", [D, 128], bf16, kind="ExternalInput").ap()
    wqb_ap = nc.dram_tensor("wqb", [D, 128], bf16, kind="ExternalInput").ap()
    wv_ap = nc.dram_tensor("wv", [D, 128], bf16, kind="ExternalInput").ap()
    wo_ap = nc.dram_tensor("wo", [128, D], bf16, kind="ExternalInput").ap()
    ba_ap = nc.dram_tensor("ba", [128, 1], f32, kind="ExternalInput").ap()
    bb_ap = nc.dram_tensor("bb", [128, 1], f32, kind="ExternalInput").ap()
    bv_ap = nc.dram_tensor("bv", [128, 1], f32, kind="ExternalInput").ap()
    ones_ap = nc.dram_tensor("ones", [128, 64], bf16, kind="ExternalInput").ap()
    ident_ap = nc.dram_tensor("ident", [128, 128], bf16,
                              kind="ExternalInput").ap()
    out_ap = nc.dram_tensor("partial", [D, M], bf16,
                            kind="ExternalOutput").ap()

    with tile.TileContext(nc) as tc:
        with tc.tile_pool(name="persist", bufs=1) as persist, \
             tc.tile_pool(name="const", bufs=1) as const, \
             tc.tile_pool(name="xpool", bufs=1) as xpool, \
             tc.tile_pool(name="vt_pool", bufs=2) as vt_pool, \
             tc.tile_pool(name="ps8", bufs=1, space="PSUM") as ps8, \
             tc.tile_pool(name="epool", bufs=1) as epool, \
             tc.tile_pool(name="stage", bufs=2) as stage, \
             tc.tile_pool(name="fin", bufs=4) as fin:
            qka_sb = persist.tile([128, M], bf16, tag="qka")
            qkb_sb = persist.tile([128, M], bf16, tag="qkb")
            v_sb = persist.tile([128, B, N_KKTILE, HEADS_PER_CORE, 65], bf16,
                                tag="vsb")
            outt_sb = persist.tile([128, M], bf16, tag="outt")

            # DMA order is tuned so the head's critical inputs land first:
            # K weights (scalar), x chunk 0 (sync+gpsimd), then V/Q weights,
            # small constants, the rest of x, and finally wo.
            wq_sb = const.tile([128, 3, N_KTILE, 128], bf16, tag="wq")
            ident_sb = const.tile([128, 128], bf16, tag="ident")
            nc.scalar.dma_start(ident_sb[:], ident_ap[:])
            bb_sb = const.tile([128, 1], f32, tag="bb")
            nc.scalar.dma_start(bb_sb[:], bb_ap[:])
            for ki in range(N_KTILE):
                nc.scalar.dma_start(wq_sb[:, 1, ki],
                                    wqb_ap[ki * 128:(ki + 1) * 128, :])
            xs = xpool.tile([128, B * N_MCHUNK_B, N_KTILE, 512], bf16,
                            tag="xs")

            def x_dma(mi, ki, eng):
                eng.dma_start(
                    xs[:, mi, ki],
                    xt_ap[ki * 128:(ki + 1) * 128, mi * 512:(mi + 1) * 512])

            for ki in range(N_KTILE):
                x_dma(0, ki, (nc.sync, nc.gpsimd)[ki % 2])
            for ki in range(N_KTILE):
                nc.gpsimd.dma_start(wq_sb[:, 2, ki],
                                    wv_ap[ki * 128:(ki + 1) * 128, :])
                nc.sync.dma_start(wq_sb[:, 0, ki],
                                  wqa_ap[ki * 128:(ki + 1) * 128, :])
            ba_sb = const.tile([128, 1], f32, tag="ba")
            nc.scalar.dma_start(ba_sb[:], ba_ap[:])
            bv_sb = const.tile([128, 1], f32, tag="bv")
            nc.scalar.dma_start(bv_sb[:], bv_ap[:])
            ones_sb = const.tile([128, 64], bf16, tag="ones")
            nc.scalar.dma_start(ones_sb[:], ones_ap[:])
            wo_sb = const.tile([128, D], bf16, tag="wo")
            # ones column of v_sb: a DMA here is a 2-byte-element scatter
            # that stalls the queue for ~13us; memset is ~free on DVE
            nc.vector.memset(v_sb[:, :, :, :, 64:65], 1.0)
            n = 0
            for mi in range(1, B * N_MCHUNK_B):
                for ki in range(N_KTILE):
                    x_dma(mi, ki, (nc.sync, nc.gpsimd)[n % 2])
                    n += 1
            # wo isn't needed until the first out-projection (~100us in)
            nc.sync.dma_start(wo_sb[:], wo_ap[:])

            vts = [None, None]
            # SBUF->SBUF DMA writes (outt partitions 64:128) are not seen by
            # the dependency tracker; record them and pin explicit sync edges
            # from the out-projection matmuls that read them.
            outt_h1_dma = {}

            def qkv_chunk_items(b2, ei, mc, vt_sb):
                """One projection m-chunk: 8 accumulating matmuls + bias
                eviction (+ V transposes), one yielded closure each."""
                bias, dest, dcol = (
                    (ba_sb, qka_sb, b2 * S),
                    (bb_sb, qkb_sb, b2 * S),
                    (bv_sb, vt_sb, 0),
                )[ei]
                mi = b2 * N_MCHUNK_B + mc
                ps = ps8.tile([128, 512], f32, tag="mm", bufs=2,
                              name=f"qkvps{b2}{ei}{mc}")
                for ki in range(N_KTILE):
                    def mm(ki=ki, ps=ps, ei=ei, mi=mi):
                        nc.tensor.matmul(ps[:], wq_sb[:, ei, ki],
                                         xs[:, mi, ki],
                                         start=(ki == 0),
                                         stop=(ki == N_KTILE - 1))
                    yield mm
                def evict(ps=ps, dest=dest, bias=bias, dcol=dcol, mc=mc):
                    nc.vector.tensor_scalar_add(
                        dest[:, dcol + mc * 512:dcol + (mc + 1) * 512],
                        ps[:], bias[:])
                yield evict
                if ei == 2:
                    for kt in range(mc * 4, (mc + 1) * 4):
                        def tpf(kt=kt, vt_sb=vt_sb, b2=b2):
                            tp = ps8.tile([128, 128], bf16, tag="mm", bufs=2,
                                          name=f"tp{b2}")
                            nc.tensor.transpose(
                                tp[:], vt_sb[:, kt * 128:(kt + 1) * 128],
                                ident_sb[:])
                            for h in range(HEADS_PER_CORE):
                                nc.vector.tensor_copy(
                                    v_sb[:, b2, kt, h, 0:64],
                                    tp[:, h * 64:(h + 1) * 64])
                        yield tpf

            def head_b0():
                """Minimal pre-attention work for batch 0: K chunk 0 and
                Q chunk 0 first (the first score pair only needs those),
                then V chunk 0 + its transposes for the first AV."""
                vt_sb = vt_pool.tile([128, S], bf16, tag="vt", name="vt0")
                vts[0] = vt_sb
                for ei in (1, 0, 2):
                    for item in qkv_chunk_items(0, ei, 0, vt_sb):
                        item()

            def qkv_remaining_items(b2):
                # K/V chunk-interleaved (K first within each pair keeps the
                # score inputs ahead of the kt loop), then Q chunks 1-3;
                # item-count layout matches _KEND/_VEND/_QEND below.
                for mc in range(1, N_MCHUNK_B):
                    yield from qkv_chunk_items(b2, 1, mc, vts[b2])
                    yield from qkv_chunk_items(b2, 2, mc, vts[b2])
                for mc in range(1, N_MCHUNK_B):
                    yield from qkv_chunk_items(b2, 0, mc, vts[b2])

            def b0_items():
                yield from qkv_remaining_items(0)
                # b1's K/V (and Q chunk 0) are produced across b0's window
                # (2-3 items/slot is sustainable; packing them into b1's own
                # qi0 would need ~5.5/slot and starve the exp stream). Q
                # chunks 1-3 of b1 are deferred to b1's own window since
                # they aren't read until its qi 1-3.
                vt_sb = vt_pool.tile([128, S], bf16, tag="vt", name="vt1")
                vts[1] = vt_sb
                yield from qkv_chunk_items(1, 1, 0, vt_sb)
                yield from qkv_chunk_items(1, 2, 0, vt_sb)
                yield from qkv_chunk_items(1, 0, 0, vt_sb)
                for mc in range(1, N_MCHUNK_B):
                    yield from qkv_chunk_items(1, 1, mc, vt_sb)
                    yield from qkv_chunk_items(1, 2, mc, vt_sb)

            def q123_b1_items():
                for mc in range(1, N_MCHUNK_B):
                    yield from qkv_chunk_items(1, 0, mc, vts[1])

            # emitted-item prerequisites for attn(b0): the kt loop must not
            # be EMITTED past injected producers it reads (emission order is
            # queue order; a read emitted before its writer is a race).
            # item ends: K1=9 V1=22 K2=31 V2=44 K3=53 V3=66 Q1=75 Q2=84 Q3=93
            _KEND = {0: 0, 1: 9, 2: 31, 3: 53}
            _VEND = {0: 0, 1: 22, 2: 44, 3: 66}
            _QEND = {0: 0, 1: 75, 2: 84, 3: 93}

            def b0_sc_prereq(qi, kt):
                return max(_KEND[kt // 4], _QEND[qi])

            def b0_av_prereq(qi, kt):
                return _VEND[kt // 4]

            def outproj_items(b2, mcs):
                for mc in mcs:
                    mrow = b2 * S + mc * 512
                    for et in range(D // 128):
                        def mm(et=et, mrow=mrow, b2=b2, mc=mc):
                            fp = ps8.tile([128, 512], f32, tag="mm", bufs=2,
                                          name="fp")
                            mi_ = nc.tensor.matmul(
                                fp[:], wo_sb[:, et * 128:(et + 1) * 128],
                                outt_sb[:, mrow:mrow + 512],
                                start=True, stop=True)
                            _add_dep_helper(
                                mi_.ins, outt_h1_dma[(b2, mc)].ins, sync=True,
                                reason="outt[64:128] RAW on sb2sb dma")
                            fo = fin.tile([128, 512], bf16, tag="fo",
                                          name="fo")
                            nc.vector.tensor_copy(fo[:], fp[:])
                            eng = (nc.sync, nc.gpsimd)[et % 2]
                            if b2 == 1 and mc == 3:
                                eng = nc.sync
                            eng.dma_start(
                                out_ap[et * 128:(et + 1) * 128,
                                       mrow:mrow + 512],
                                fo[:])
                        yield mm

            def attn_phase(b2, buckets, budgets, sc_prereq=None,
                           av_prereq=None):
                """Attention for batch b2. After each score pair, pull up to
                budgets[qi] items of other-phase PE work from `buckets`
                (list of [start_qi, generator]; items of a bucket may only
                be emitted from its start_qi on, so consumers never get
                emitted ahead of their in-phase producers). sc/av_prereq
                give the minimum bucket-0 item count that must be emitted
                before the score pair / AV pair of a slot."""
                pulled0 = [0]
                cur_qi = [0]

                cur_kt = [0]

                def draw_one():
                    for start_qi, gen in buckets:
                        # a bucket opens one slot into its qi: the eviction
                        # whose output it reads is emitted in slot (qi, 1)
                        if (start_qi, 1) > (cur_qi[0], cur_kt[0]):
                            continue
                        item = next(gen, None)
                        if item is not None:
                            if gen is buckets[0][1]:
                                pulled0[0] += 1
                            item()
                            return True
                    return False

                def pull(n):
                    for _ in range(n):
                        if not draw_one():
                            return

                def pull_to(n):
                    gen = buckets[0][1]
                    while pulled0[0] < n:
                        item = next(gen, None)
                        if item is None:
                            return
                        pulled0[0] += 1
                        item()

                pending_evict = [None]

                def make_evict(qi, avp, qcol):
                    def evict():
                        # normalization: broadcast the ones-column sums (PSUM
                        # partition 64) down to 0..63 with a 1-row matmul,
                        # then reciprocal + multiply straight out of PSUM.
                        # Emitted AFTER the next qi's first score pair + exp
                        # so the broadcast matmuls never gate the exp stream.
                        stbs, rbps, rb2s = [], [], []
                        for h in (0, 1):
                            stb = stage.tile([128, 512], bf16, tag="stb",
                                             name="stb")
                            nc.vector.tensor_copy(stb[64:65, :],
                                                  avp[h][64:65, :])
                            stbs.append(stb)
                        for h in (0, 1):
                            rbp = ps8.tile([128, 512], f32, tag="mm", bufs=2,
                                           name="rbp")
                            nc.tensor.matmul(rbp[0:64, :],
                                             ones_sb[64:65, 0:64],
                                             stbs[h][64:65, :],
                                             start=True, stop=True)
                            rbps.append(rbp)
                        for h in (0, 1):
                            rb2 = stage.tile([128, 512], f32, tag="rb2",
                                             name="rb2")
                            nc.vector.reciprocal_approx_fast(rb2[0:64, :],
                                                             rbps[h][0:64, :])
                            rb2s.append(rb2)
                        nc.vector.tensor_mul(outt_sb[0:64, qcol:qcol + 512],
                                             avp[0][0:64, :], rb2s[0][0:64, :])
                        tm = stage.tile([128, 512], bf16, tag="tm", name="tm")
                        nc.vector.tensor_mul(tm[0:64, :], avp[1][0:64, :],
                                             rb2s[1][0:64, :])
                        outt_h1_dma[(b2, qi)] = nc.gpsimd.dma_start(
                            outt_sb[64:128, qcol:qcol + 512], tm[0:64, :])
                    return evict

                for qi in range(N_QCHUNK):
                    cur_qi[0] = qi
                    qcol = b2 * S + qi * 512
                    avp = [ps8.tile([128, 512], f32, tag=f"av{h}",
                                    name=f"av{h}")
                           for h in range(HEADS_PER_CORE)]
                    pending = None

                    def emit_av(kt, e, avp=avp):
                        first = (kt == 0)
                        last = (kt == N_KKTILE - 1)
                        for h in range(HEADS_PER_CORE):
                            nc.tensor.matmul(
                                avp[h][0:65, :],
                                v_sb[:, b2, kt, h, :],
                                e[:, h * 512:(h + 1) * 512],
                                start=first, stop=last)

                    for kt in range(N_KKTILE):
                        cur_kt[0] = kt
                        kkcol = b2 * S + kt * 128
                        if sc_prereq is not None:
                            pull_to(sc_prereq(qi, kt))
                        sc = ps8.tile([128, 1024], f32, tag="sc", bufs=2,
                                      name="sc")
                        for h in range(HEADS_PER_CORE):
                            nc.tensor.matmul(
                                sc[:, h * 512:(h + 1) * 512],
                                qkb_sb[h * 64:(h + 1) * 64, kkcol:kkcol + 128],
                                qka_sb[h * 64:(h + 1) * 64, qcol:qcol + 512],
                                start=True, stop=True)
                        e = epool.tile([128, 1024], bf16, tag="e", bufs=4,
                                       name="e")
                        nc.scalar.activation(e[:], sc[:], Exp, scale=SCALE)
                        if pending_evict[0] is not None:
                            pending_evict[0]()
                            pending_evict[0] = None
                        pull(budgets[qi])
                        if pending is not None:
                            if av_prereq is not None:
                                pull_to(av_prereq(qi, pending[0]))
                            emit_av(*pending)
                        pending = (kt, e)
                    if av_prereq is not None:
                        pull_to(av_prereq(qi, pending[0]))
                    emit_av(*pending)
                    pending_evict[0] = make_evict(qi, avp, qcol)
                # final eviction, then whatever the slots didn't absorb
                pending_evict[0]()
                pending_evict[0] = None
                cur_qi[0] = N_QCHUNK
                cur_kt[0] = 0
                pull(1 << 30)

            # ~3us of dummy transposes warm the PE clock (HAM 4/8 -> 8/8)
            # while the x DMAs land, so the real head runs at 2.4 GHz
            for w in range(24):
                wt = ps8.tile([128, 128], bf16, tag="sc", bufs=2,
                              name="warm")
                nc.tensor.transpose(wt[:], ident_sb[:], ident_sb[:])

            head_b0()
            attn_phase(0, [[0, b0_items()],
                           [1, outproj_items(0, [0])]],
                       budgets=(4, 4, 3, 3),
                       sc_prereq=b0_sc_prereq, av_prereq=b0_av_prereq)

            _QB1 = {0: 0, 1: 9, 2: 18, 3: 27}
            attn_phase(1, [[0, q123_b1_items()],
                           [0, outproj_items(0, [3])],
                           [0, outproj_items(0, [1])],
                           [0, outproj_items(0, [2])],
                           [1, outproj_items(1, [0])],
                           [2, outproj_items(1, [1])],
                           [3, outproj_items(1, [2])]],
                       budgets=(1, 2, 2, 2),
                       sc_prereq=lambda qi, kt: _QB1[qi])
            for w in range(8):
                wt = ps8.tile([128, 128], bf16, tag="sc", bufs=2,
                              name="warmtail")
                nc.tensor.transpose(wt[:], ident_sb[:], ident_sb[:])
            for item in outproj_items(1, [3]):
                item()
    nc.compile()
    return nc


def _shard_inputs(x, w_qkv, b_qkv, w_out):
    import ml_dtypes
    bf16 = ml_dtypes.bfloat16
    xt = np.ascontiguousarray(x.reshape(M, D).T.astype(bf16))  # (1024, 4096)
    ones = np.ones((128, 64), dtype=bf16)
    ident = np.eye(128, dtype=bf16)
    in_maps = []
    for c in range(N_CORES):
        h0 = HEADS_PER_CORE * c
        rows_q, rows_k, rows_v, dcols = [], [], [], []
        for h in (h0, h0 + 1):
            rows_q += list(range(h * 192, h * 192 + 64))
            rows_k += list(range(h * 192 + 64, h * 192 + 128))
            rows_v += list(range(h * 192 + 128, h * 192 + 192))
            dcols += list(range(h * 64, (h + 1) * 64))
        in_maps.append({
            "xt": xt,
            "wqa": np.ascontiguousarray(w_qkv[rows_q, :].T.astype(bf16)),
            "wqb": np.ascontiguousarray(w_qkv[rows_k, :].T.astype(bf16)),
            "wv": np.ascontiguousarray(w_qkv[rows_v, :].T.astype(bf16)),
            "wo": np.ascontiguousarray(w_out[:, dcols].T.astype(bf16)),
            "ba": np.ascontiguousarray(b_qkv[rows_q].reshape(128, 1)),
            "bb": np.ascontiguousarray(b_qkv[rows_k].reshape(128, 1)),
            "bv": np.ascontiguousarray(b_qkv[rows_v].reshape(128, 1)),
            "ones": ones,
            "ident": ident,
        })
    return in_maps


def kernel(x, w_qkv, b_qkv, w_out, b_out, _trace=False):
    from concourse.bass_utils import run_bass_kernel_spmd

    x = np.asarray(x, dtype=np.float32)
    w_qkv = np.asarray(w_qkv, dtype=np.float32)
    b_qkv = np.asarray(b_qkv, dtype=np.float32)
    w_out = np.asarray(w_out, dtype=np.float32)
    b_out = np.asarray(b_out, dtype=np.float32)

    if "nc" not in _CACHE:
        _CACHE["nc"] = _build_module()
    nc = _CACHE["nc"]

    in_maps = _shard_inputs(x, w_qkv, b_qkv, w_out)
    res = run_bass_kernel_spmd(nc, in_maps, list(range(N_CORES)), trace=_trace)
    acc = np.zeros((D, M), dtype=np.float64)
    for c in range(N_CORES):
        acc += np.asarray(res.results[c]["partial"], dtype=np.float64)
    acc = acc.T + b_out
    out = acc.astype(np.float32).reshape(B, S, D)
    if _trace:
        _CACHE["last_exec_time_ns"] = res.exec_time_ns
        _CACHE["last_res"] = res
    return out
